# revision 6
# baseline (speedup 1.0000x reference)
"""Causal multi-head attention on 8 Trainium2 NeuronCores.

Full inputs q/k/v: [2, 16, 2048, 64] f32. The 32 (batch, head) slices are
sharded 4-per-core across 8 cores; each core runs an SPMD Bass/Tile kernel
computing causal softmax(Q K^T / 8) V for its 4 slices.

Self-contained: only needs numpy + the concourse runtime at /opt/trn_rl_repo.
"""

import os
import sys

import numpy as np

for _p in ("/opt/trn_rl_repo",):
    if _p not in sys.path:
        sys.path.insert(0, _p)

from concourse import bass, tile, mybir  # noqa: E402
from concourse.bass_utils import run_bass_kernel_spmd  # noqa: E402

F32 = mybir.dt.float32

B, H, SEQ, D = 2, 16, 2048, 64
N_CORES = 8
SLICES = (B * H) // N_CORES  # 4 per core
NBLK = SEQ // 128  # 16 seq blocks of 128
HALF = 1024  # q processed in halves (PSUM budget)
EXP = mybir.ActivationFunctionType.Exp

NEG_BIG = -1.0e30


def _pieces(start_col):
    """Split [start_col, 1024) at the 512 PSUM-bank boundary."""
    if start_col < 512:
        return [(start_col, 512), (512, 1024)]
    if start_col < 1024:
        return [(start_col, 1024)]
    return []


def _attention_body(ctx, tc, q, k, v, ident, mask, out, slices=SLICES):
    nc = tc.nc

    const_pool = ctx.enter_context(tc.tile_pool(name="const", bufs=1))
    nat_pool = ctx.enter_context(tc.tile_pool(name="nat", bufs=2))
    qkt_pool = ctx.enter_context(tc.tile_pool(name="qkt", bufs=2))
    vt_pool = ctx.enter_context(tc.tile_pool(name="vt", bufs=2))
    pt_pool = ctx.enter_context(tc.tile_pool(name="pt", bufs=3))
    ots_pool = ctx.enter_context(tc.tile_pool(name="ots", bufs=2))
    osb_pool = ctx.enter_context(tc.tile_pool(name="osb", bufs=2))
    rec_pool = ctx.enter_context(tc.tile_pool(name="rec", bufs=2))
    st_psum = ctx.enter_context(tc.tile_pool(name="stp", bufs=2, space="PSUM"))
    ot_psum = ctx.enter_context(tc.tile_pool(name="otp", bufs=1, space="PSUM"))
    tr_psum = ctx.enter_context(tc.tile_pool(name="trp", bufs=2, space="PSUM"))

    ident_sb = const_pool.tile([128, 128], F32)
    nc.sync.dma_start(ident_sb[:, :], ident[:, :])
    mask_sb = const_pool.tile([128, 128], F32)
    nc.sync.dma_start(mask_sb[:, :], mask[:, :])

    for s in range(slices):
        # ---- load this slice ----
        qn = nat_pool.tile([128, NBLK, 64], F32, tag="qn")
        nc.sync.dma_start(qn[:, :, :], q[s].rearrange("(n p) d -> p n d", p=128))
        kn = nat_pool.tile([128, NBLK, 64], F32, tag="kn")
        nc.sync.dma_start(kn[:, :, :], k[s].rearrange("(n p) d -> p n d", p=128))
        vt = vt_pool.tile([128, NBLK, 65], F32)
        nc.sync.dma_start(vt[:, :, 0:64], v[s].rearrange("(n p) d -> p n d", p=128))
        nc.vector.memset(vt[:, :, 64:65], 1.0)  # ones col -> softmax denominator

        # ---- build Q^T, K^T [64, 2048] via PE transposes ----
        qt = qkt_pool.tile([64, SEQ], F32, tag="qt")
        kt = qkt_pool.tile([64, SEQ], F32, tag="kt")
        for nat, tt in ((qn, qt), (kn, kt)):
            for c in range(4):  # 512-wide chunks
                tp = tr_psum.tile([64, 512], F32, tag="tr")
                for j in range(4):
                    nc.tensor.transpose(
                        tp[:, j * 128 : (j + 1) * 128],
                        nat[:, 4 * c + j, :],
                        ident_sb[:, :],
                    )
                nc.vector.tensor_copy(tt[:, c * 512 : (c + 1) * 512], tp[:, :])

        # ---- main loop: q in two halves ----
        for h in range(2):
            ot = ot_psum.tile([65, HALF], F32, tag="ot")
            kb_hi = 8 if h == 0 else 16
            for kb in range(kb_hi):
                start_col = max(0, 128 * kb - HALF * h)
                pieces = _pieces(start_col)

                # S^T[kb, q] = K[kb] @ Q^T   (contraction over d=64)
                st = st_psum.tile([128, HALF], F32, tag="st")
                for a, b in pieces:
                    nc.tensor.matmul(
                        st[:, a:b],
                        kt[:, 128 * kb : 128 * kb + 128],
                        qt[:, HALF * h + a : HALF * h + b],
                        start=True,
                        stop=True,
                    )
                # causal mask on the diagonal block
                if 8 * h <= kb < 8 * h + 8:
                    dc = 128 * kb - HALF * h
                    nc.vector.tensor_add(
                        st[:, dc : dc + 128], st[:, dc : dc + 128], mask_sb[:, :]
                    )
                # P^T = exp(S^T / 8)
                pt = pt_pool.tile([128, HALF], F32, tag="pt")
                nc.scalar.activation(
                    pt[:, start_col:HALF],
                    st[:, start_col:HALF],
                    EXP,
                    bias=0.0,
                    scale=0.125,
                )
                # O^T[d|1, q] += [V[kb] | 1]^T.T @ P^T
                for a, b in pieces:
                    last_kb = 8 * h + 3 if b == 512 else 8 * h + 7
                    nc.tensor.matmul(
                        ot[:, a:b],
                        vt[:, kb, :],
                        pt[:, a:b],
                        start=(kb == 0),
                        stop=(kb == last_kb),
                    )

            # ---- epilogue: transpose back, normalize, store ----
            ots = ots_pool.tile([65, HALF], F32)
            nc.vector.tensor_copy(ots[:, :], ot[:, :])
            outt = st_psum.tile([128, 8, 128], F32, tag="st")
            for j in range(8):
                nc.tensor.transpose(
                    outt[:, j, 0:65],
                    ots[:, j * 128 : (j + 1) * 128],
                    ident_sb[0:65, 0:65],
                )
            rec = rec_pool.tile([128, 8], F32)
            nc.vector.reciprocal(rec[:, :], outt[:, :, 64])
            osb = osb_pool.tile([128, 8, 64], F32)
            nc.vector.tensor_mul(
                osb[:, :, :], outt[:, :, 0:64], rec[:, :].broadcast_to([128, 8, 64])
            )
            nc.sync.dma_start(
                out[s].rearrange("(n p) d -> p n d", p=128)[:, 8 * h : 8 * h + 8, :],
                osb[:, :, :],
            )


def build_nc(slices=SLICES):
    nc = bass.Bass()
    q = nc.declare_dram_parameter("q", [slices, SEQ, D], F32, isOutput=False)
    k = nc.declare_dram_parameter("k", [slices, SEQ, D], F32, isOutput=False)
    v = nc.declare_dram_parameter("v", [slices, SEQ, D], F32, isOutput=False)
    ident = nc.declare_dram_parameter("ident", [128, 128], F32, isOutput=False)
    mask = nc.declare_dram_parameter("mask", [128, 128], F32, isOutput=False)
    out = nc.declare_dram_parameter("out", [slices, SEQ, D], F32, isOutput=True)

    import contextlib

    with tile.TileContext(nc) as tc:
        with contextlib.ExitStack() as ctx:
            _attention_body(
                ctx, tc, q[:], k[:], v[:], ident[:], mask[:], out[:], slices=slices
            )
    # TRN2 allows at most one sync-wait per instruction; Tile can emit more.
    # These bacc legalization passes split the excess onto ldweights /
    # InstEventSemaphore instructions so walrus codegen accepts the BIR.
    import bass_rust

    bass_rust.move_matmul_waits_to_ldweights(nc.m)
    bass_rust.generate_event_semaphores(nc)
    return nc


def _consts():
    ident = np.eye(128, dtype=np.float32)
    i = np.arange(128, dtype=np.int64)
    mask = np.where(i[:, None] <= i[None, :], 0.0, NEG_BIG).astype(np.float32)
    return ident, mask


_NC_CACHE = {}


def _get_nc(slices=SLICES):
    if slices not in _NC_CACHE:
        _NC_CACHE[slices] = build_nc(slices)
    return _NC_CACHE[slices]


def run_sharded(q, k, v, trace=False):
    """Run on 8 cores. Returns (out_full, exec_time_ns_or_None)."""
    nc = _get_nc()
    ident, mask = _consts()
    qs = np.ascontiguousarray(np.asarray(q, dtype=np.float32).reshape(B * H, SEQ, D))
    ks = np.ascontiguousarray(np.asarray(k, dtype=np.float32).reshape(B * H, SEQ, D))
    vs = np.ascontiguousarray(np.asarray(v, dtype=np.float32).reshape(B * H, SEQ, D))
    in_maps = []
    for c in range(N_CORES):
        sl = slice(SLICES * c, SLICES * (c + 1))
        in_maps.append(
            {
                "q": qs[sl],
                "k": ks[sl],
                "v": vs[sl],
                "ident": ident,
                "mask": mask,
            }
        )
    res = run_bass_kernel_spmd(nc, in_maps, list(range(N_CORES)), trace=trace)
    outs = [res.results[c]["out"] for c in range(N_CORES)]
    full = np.concatenate(outs, axis=0).reshape(B, H, SEQ, D)
    return full, res.exec_time_ns


def kernel(q, k, v):
    out, _ = run_sharded(q, k, v, trace=False)
    return out


# revision 21
# speedup vs baseline: 1.4865x; 1.4865x over previous
"""Causal multi-head attention on 8 Trainium2 NeuronCores.

Full inputs q/k/v: [2, 16, 2048, 64] f32. The 32 (batch, head) slices are
sharded 4-per-core across 8 cores; each core runs an SPMD Bass/Tile kernel
computing causal softmax(Q K^T / 8) V for its 4 slices.

Self-contained: only needs numpy + the concourse runtime at /opt/trn_rl_repo.
"""

import os
import sys

import numpy as np

for _p in ("/opt/trn_rl_repo",):
    if _p not in sys.path:
        sys.path.insert(0, _p)

from concourse import bass, tile, mybir  # noqa: E402
from concourse.bass_utils import run_bass_kernel_spmd  # noqa: E402

F32 = mybir.dt.float32
F32R = mybir.dt.float32r  # fp32 bits, single-pass PE matmul (4x faster)

B, H, SEQ, D = 2, 16, 2048, 64
N_CORES = 8
SLICES = (B * H) // N_CORES  # 4 per core
NBLK = SEQ // 128  # 16 seq blocks of 128
HALF = 1024  # q processed in halves (PSUM budget)
EXP = mybir.ActivationFunctionType.Exp

NEG_BIG = -1.0e30


def _pieces(start_col):
    """Split [start_col, 1024) at the 512 PSUM-bank boundary."""
    if start_col < 512:
        return [(start_col, 512), (512, 1024)]
    if start_col < 1024:
        return [(start_col, 1024)]
    return []


def _attention_body(ctx, tc, q, k, v, ident, mask, ones, out, slices=SLICES):
    nc = tc.nc

    const_pool = ctx.enter_context(tc.tile_pool(name="const", bufs=1))
    nat_pool = ctx.enter_context(tc.tile_pool(name="nat", bufs=2))
    qkt_pool = ctx.enter_context(tc.tile_pool(name="qkt", bufs=2))
    vt_pool = ctx.enter_context(tc.tile_pool(name="vt", bufs=2))
    pt_pool = ctx.enter_context(tc.tile_pool(name="pt", bufs=3))
    ots_pool = ctx.enter_context(tc.tile_pool(name="ots", bufs=2))
    osb_pool = ctx.enter_context(tc.tile_pool(name="osb", bufs=2))
    rec_pool = ctx.enter_context(tc.tile_pool(name="rec", bufs=2))
    st_psum = ctx.enter_context(tc.tile_pool(name="stp", bufs=2, space="PSUM"))
    ot_psum = ctx.enter_context(tc.tile_pool(name="otp", bufs=1, space="PSUM"))
    tr_psum = ctx.enter_context(tc.tile_pool(name="trp", bufs=2, space="PSUM"))

    ident_sb = const_pool.tile([128, 128], F32)
    nc.sync.dma_start(ident_sb[:, :], ident[:, :])
    mask_sb = const_pool.tile([128, 128], F32)
    nc.sync.dma_start(mask_sb[:, :], mask[:, :])

    for s in range(slices):
        # ---- load this slice ----
        qn = nat_pool.tile([128, NBLK, 64], F32, tag="qn")
        nc.sync.dma_start(qn[:, :, :], q[s].rearrange("(n p) d -> p n d", p=128))
        kn = nat_pool.tile([128, NBLK, 64], F32, tag="kn")
        nc.sync.dma_start(kn[:, :, :], k[s].rearrange("(n p) d -> p n d", p=128))
        vt = vt_pool.tile([128, NBLK, 65], F32R)
        # SWDGE (gpsimd) path: casts f32 -> f32r during the DMA
        nc.gpsimd.dma_start(vt[:, :, 0:64], v[s].rearrange("(n p) d -> p n d", p=128))
        nc.gpsimd.dma_start(vt[:, :, 64:65], ones[:, :, :])  # -> softmax denominator

        # ---- build Q^T, K^T [64, 2048] via PE transposes ----
        qt = qkt_pool.tile([64, SEQ], F32R, tag="qt")
        kt = qkt_pool.tile([64, SEQ], F32R, tag="kt")
        for nat, tt in ((qn, qt), (kn, kt)):
            for c in range(4):  # 512-wide chunks
                tp = tr_psum.tile([64, 512], F32, tag="tr")
                for j in range(4):
                    nc.tensor.transpose(
                        tp[:, j * 128 : (j + 1) * 128],
                        nat[:, 4 * c + j, :],
                        ident_sb[:, :],
                    )
                nc.vector.tensor_copy(tt[:, c * 512 : (c + 1) * 512], tp[:, :])

        # ---- main loop: q in two halves ----
        for h in range(2):
            ot = ot_psum.tile([65, HALF], F32, tag="ot")
            kb_hi = 8 if h == 0 else 16
            for kb in range(kb_hi):
                start_col = max(0, 128 * kb - HALF * h)
                pieces = _pieces(start_col)

                # S^T[kb, q] = K[kb] @ Q^T   (contraction over d=64)
                st = st_psum.tile([128, HALF], F32, tag="st")
                for a, b in pieces:
                    nc.tensor.matmul(
                        st[:, a:b],
                        kt[:, 128 * kb : 128 * kb + 128],
                        qt[:, HALF * h + a : HALF * h + b],
                        start=True,
                        stop=True,
                    )
                # causal mask on the diagonal block
                if 8 * h <= kb < 8 * h + 8:
                    dc = 128 * kb - HALF * h
                    nc.vector.tensor_add(
                        st[:, dc : dc + 128], st[:, dc : dc + 128], mask_sb[:, :]
                    )
                # P^T = exp(S^T / 8)
                pt = pt_pool.tile([128, HALF], F32R, tag="pt")
                nc.scalar.activation(
                    pt[:, start_col:HALF],
                    st[:, start_col:HALF],
                    EXP,
                    bias=0.0,
                    scale=0.125,
                )
                # O^T[d|1, q] += [V[kb] | 1]^T.T @ P^T
                for a, b in pieces:
                    last_kb = 8 * h + 3 if b == 512 else 8 * h + 7
                    nc.tensor.matmul(
                        ot[:, a:b],
                        vt[:, kb, :],
                        pt[:, a:b],
                        start=(kb == 0),
                        stop=(kb == last_kb),
                    )

            # ---- epilogue: transpose back, normalize, store ----
            ots = ots_pool.tile([65, HALF], F32)
            nc.vector.tensor_copy(ots[:, :], ot[:, :])
            outt = st_psum.tile([128, 8, 128], F32, tag="st")
            for j in range(8):
                nc.tensor.transpose(
                    outt[:, j, 0:65],
                    ots[:, j * 128 : (j + 1) * 128],
                    ident_sb[0:65, 0:65],
                )
            rec = rec_pool.tile([128, 8], F32)
            nc.vector.reciprocal(rec[:, :], outt[:, :, 64])
            osb = osb_pool.tile([128, 8, 64], F32)
            nc.vector.tensor_mul(
                osb[:, :, :], outt[:, :, 0:64], rec[:, :].broadcast_to([128, 8, 64])
            )
            nc.sync.dma_start(
                out[s].rearrange("(n p) d -> p n d", p=128)[:, 8 * h : 8 * h + 8, :],
                osb[:, :, :],
            )


def build_nc(slices=SLICES):
    nc = bass.Bass()
    q = nc.declare_dram_parameter("q", [slices, SEQ, D], F32, isOutput=False)
    k = nc.declare_dram_parameter("k", [slices, SEQ, D], F32, isOutput=False)
    v = nc.declare_dram_parameter("v", [slices, SEQ, D], F32, isOutput=False)
    ident = nc.declare_dram_parameter("ident", [128, 128], F32, isOutput=False)
    mask = nc.declare_dram_parameter("mask", [128, 128], F32, isOutput=False)
    ones = nc.declare_dram_parameter("ones", [128, NBLK, 1], F32, isOutput=False)
    out = nc.declare_dram_parameter("out", [slices, SEQ, D], F32, isOutput=True)

    import contextlib

    with tile.TileContext(nc) as tc:
        with contextlib.ExitStack() as ctx:
            _attention_body(
                ctx,
                tc,
                q[:],
                k[:],
                v[:],
                ident[:],
                mask[:],
                ones[:],
                out[:],
                slices=slices,
            )
    # TRN2 allows at most one sync-wait per instruction; Tile can emit more.
    # These bacc legalization passes split the excess onto ldweights /
    # InstEventSemaphore instructions so walrus codegen accepts the BIR.
    import bass_rust

    bass_rust.move_matmul_waits_to_ldweights(nc.m)
    bass_rust.generate_event_semaphores(nc)
    return nc


def _consts():
    ident = np.eye(128, dtype=np.float32)
    i = np.arange(128, dtype=np.int64)
    mask = np.where(i[:, None] <= i[None, :], 0.0, NEG_BIG).astype(np.float32)
    ones = np.ones((128, NBLK, 1), dtype=np.float32)
    return ident, mask, ones


_NC_CACHE = {}


def _get_nc(slices=SLICES):
    if slices not in _NC_CACHE:
        _NC_CACHE[slices] = build_nc(slices)
    return _NC_CACHE[slices]


def run_sharded(q, k, v, trace=False):
    """Run on 8 cores. Returns (out_full, exec_time_ns_or_None)."""
    nc = _get_nc()
    ident, mask, ones = _consts()
    qs = np.ascontiguousarray(np.asarray(q, dtype=np.float32).reshape(B * H, SEQ, D))
    ks = np.ascontiguousarray(np.asarray(k, dtype=np.float32).reshape(B * H, SEQ, D))
    vs = np.ascontiguousarray(np.asarray(v, dtype=np.float32).reshape(B * H, SEQ, D))
    in_maps = []
    for c in range(N_CORES):
        sl = slice(SLICES * c, SLICES * (c + 1))
        in_maps.append(
            {
                "q": qs[sl],
                "k": ks[sl],
                "v": vs[sl],
                "ident": ident,
                "mask": mask,
                "ones": ones,
            }
        )
    res = run_bass_kernel_spmd(nc, in_maps, list(range(N_CORES)), trace=trace)
    outs = [res.results[c]["out"] for c in range(N_CORES)]
    full = np.concatenate(outs, axis=0).reshape(B, H, SEQ, D)
    return full, res.exec_time_ns


def kernel(q, k, v):
    out, _ = run_sharded(q, k, v, trace=False)
    return out


# revision 24
# speedup vs baseline: 1.4878x; 1.0009x over previous
"""Causal multi-head attention on 8 Trainium2 NeuronCores.

Full inputs q/k/v: [2, 16, 2048, 64] f32. The 32 (batch, head) slices are
sharded 4-per-core across 8 cores; each core runs an SPMD Bass/Tile kernel
computing causal softmax(Q K^T / 8) V for its 4 slices.

Self-contained: only needs numpy + the concourse runtime at /opt/trn_rl_repo.
"""

import os
import sys

import numpy as np

for _p in ("/opt/trn_rl_repo",):
    if _p not in sys.path:
        sys.path.insert(0, _p)

from concourse import bass, tile, mybir  # noqa: E402
from concourse.bass_utils import run_bass_kernel_spmd  # noqa: E402

F32 = mybir.dt.float32
F32R = mybir.dt.float32r  # fp32 bits, single-pass PE matmul (4x faster)
BF16 = mybir.dt.bfloat16

B, H, SEQ, D = 2, 16, 2048, 64
N_CORES = 8
SLICES = (B * H) // N_CORES  # 4 per core
NBLK = SEQ // 128  # 16 seq blocks of 128
HALF = 1024  # q processed in halves (PSUM budget)
EXP = mybir.ActivationFunctionType.Exp

NEG_BIG = -1.0e30


def _pieces(start_col):
    """Split [start_col, 1024) at the 512 PSUM-bank boundary."""
    if start_col < 512:
        return [(start_col, 512), (512, 1024)]
    if start_col < 1024:
        return [(start_col, 1024)]
    return []


def _attention_body(ctx, tc, q, k, v, ident, mask, ones, out, slices=SLICES):
    nc = tc.nc

    const_pool = ctx.enter_context(tc.tile_pool(name="const", bufs=1))
    nat_pool = ctx.enter_context(tc.tile_pool(name="nat", bufs=2))
    qkt_pool = ctx.enter_context(tc.tile_pool(name="qkt", bufs=2))
    vt_pool = ctx.enter_context(tc.tile_pool(name="vt", bufs=2))
    pt_pool = ctx.enter_context(tc.tile_pool(name="pt", bufs=3))
    ots_pool = ctx.enter_context(tc.tile_pool(name="ots", bufs=2))
    osb_pool = ctx.enter_context(tc.tile_pool(name="osb", bufs=2))
    rec_pool = ctx.enter_context(tc.tile_pool(name="rec", bufs=2))
    st_psum = ctx.enter_context(tc.tile_pool(name="stp", bufs=2, space="PSUM"))
    ot_psum = ctx.enter_context(tc.tile_pool(name="otp", bufs=1, space="PSUM"))
    tr_psum = ctx.enter_context(tc.tile_pool(name="trp", bufs=2, space="PSUM"))

    ident_sb = const_pool.tile([128, 128], F32)
    nc.sync.dma_start(ident_sb[:, :], ident[:, :])
    mask_sb = const_pool.tile([128, 128], F32)
    nc.sync.dma_start(mask_sb[:, :], mask[:, :])

    for s in range(slices):
        # ---- load this slice ----
        qn = nat_pool.tile([128, NBLK, 64], F32, tag="qn")
        nc.sync.dma_start(qn[:, :, :], q[s].rearrange("(n p) d -> p n d", p=128))
        kn = nat_pool.tile([128, NBLK, 64], F32, tag="kn")
        nc.sync.dma_start(kn[:, :, :], k[s].rearrange("(n p) d -> p n d", p=128))
        vt = vt_pool.tile([128, NBLK, 65], BF16)
        # SWDGE (gpsimd) path: casts f32 -> bf16 during the DMA
        nc.gpsimd.dma_start(vt[:, :, 0:64], v[s].rearrange("(n p) d -> p n d", p=128))
        nc.gpsimd.dma_start(vt[:, :, 64:65], ones[:, :, :])  # -> softmax denominator

        # ---- build Q^T, K^T [64, 2048] via PE transposes ----
        qt = qkt_pool.tile([64, SEQ], F32R, tag="qt")
        kt = qkt_pool.tile([64, SEQ], F32R, tag="kt")
        for nat, tt in ((qn, qt), (kn, kt)):
            for c in range(4):  # 512-wide chunks
                tp = tr_psum.tile([64, 512], F32, tag="tr")
                for j in range(4):
                    nc.tensor.transpose(
                        tp[:, j * 128 : (j + 1) * 128],
                        nat[:, 4 * c + j, :],
                        ident_sb[:, :],
                    )
                nc.vector.tensor_copy(tt[:, c * 512 : (c + 1) * 512], tp[:, :])

        # ---- main loop: q in two halves ----
        for h in range(2):
            ot = ot_psum.tile([65, HALF], F32, tag="ot")
            kb_hi = 8 if h == 0 else 16
            for kb in range(kb_hi):
                start_col = max(0, 128 * kb - HALF * h)
                pieces = _pieces(start_col)

                # S^T[kb, q] = K[kb] @ Q^T   (contraction over d=64)
                st = st_psum.tile([128, HALF], F32, tag="st")
                for a, b in pieces:
                    nc.tensor.matmul(
                        st[:, a:b],
                        kt[:, 128 * kb : 128 * kb + 128],
                        qt[:, HALF * h + a : HALF * h + b],
                        start=True,
                        stop=True,
                    )
                # causal mask on the diagonal block
                if 8 * h <= kb < 8 * h + 8:
                    dc = 128 * kb - HALF * h
                    nc.vector.tensor_add(
                        st[:, dc : dc + 128], st[:, dc : dc + 128], mask_sb[:, :]
                    )
                # P^T = exp(S^T / 8)
                pt = pt_pool.tile([128, HALF], BF16, tag="pt")
                nc.scalar.activation(
                    pt[:, start_col:HALF],
                    st[:, start_col:HALF],
                    EXP,
                    bias=0.0,
                    scale=0.125,
                )
                # O^T[d|1, q] += [V[kb] | 1]^T.T @ P^T
                for a, b in pieces:
                    last_kb = 8 * h + 3 if b == 512 else 8 * h + 7
                    nc.tensor.matmul(
                        ot[:, a:b],
                        vt[:, kb, :],
                        pt[:, a:b],
                        start=(kb == 0),
                        stop=(kb == last_kb),
                    )

            # ---- epilogue: transpose back, normalize, store ----
            ots = ots_pool.tile([65, HALF], F32)
            nc.vector.tensor_copy(ots[:, :], ot[:, :])
            outt = st_psum.tile([128, 8, 128], F32, tag="st")
            for j in range(8):
                nc.tensor.transpose(
                    outt[:, j, 0:65],
                    ots[:, j * 128 : (j + 1) * 128],
                    ident_sb[0:65, 0:65],
                )
            rec = rec_pool.tile([128, 8], F32)
            nc.vector.reciprocal(rec[:, :], outt[:, :, 64])
            osb = osb_pool.tile([128, 8, 64], F32)
            nc.vector.tensor_mul(
                osb[:, :, :], outt[:, :, 0:64], rec[:, :].broadcast_to([128, 8, 64])
            )
            nc.sync.dma_start(
                out[s].rearrange("(n p) d -> p n d", p=128)[:, 8 * h : 8 * h + 8, :],
                osb[:, :, :],
            )


def build_nc(slices=SLICES):
    nc = bass.Bass()
    q = nc.declare_dram_parameter("q", [slices, SEQ, D], F32, isOutput=False)
    k = nc.declare_dram_parameter("k", [slices, SEQ, D], F32, isOutput=False)
    v = nc.declare_dram_parameter("v", [slices, SEQ, D], F32, isOutput=False)
    ident = nc.declare_dram_parameter("ident", [128, 128], F32, isOutput=False)
    mask = nc.declare_dram_parameter("mask", [128, 128], F32, isOutput=False)
    ones = nc.declare_dram_parameter("ones", [128, NBLK, 1], F32, isOutput=False)
    out = nc.declare_dram_parameter("out", [slices, SEQ, D], F32, isOutput=True)

    import contextlib

    with tile.TileContext(nc) as tc:
        with contextlib.ExitStack() as ctx:
            _attention_body(
                ctx,
                tc,
                q[:],
                k[:],
                v[:],
                ident[:],
                mask[:],
                ones[:],
                out[:],
                slices=slices,
            )
    # TRN2 allows at most one sync-wait per instruction; Tile can emit more.
    # These bacc legalization passes split the excess onto ldweights /
    # InstEventSemaphore instructions so walrus codegen accepts the BIR.
    import bass_rust

    bass_rust.move_matmul_waits_to_ldweights(nc.m)
    bass_rust.generate_event_semaphores(nc)
    return nc


def _consts():
    ident = np.eye(128, dtype=np.float32)
    i = np.arange(128, dtype=np.int64)
    mask = np.where(i[:, None] <= i[None, :], 0.0, NEG_BIG).astype(np.float32)
    ones = np.ones((128, NBLK, 1), dtype=np.float32)
    return ident, mask, ones


_NC_CACHE = {}


def _get_nc(slices=SLICES):
    if slices not in _NC_CACHE:
        _NC_CACHE[slices] = build_nc(slices)
    return _NC_CACHE[slices]


def run_sharded(q, k, v, trace=False):
    """Run on 8 cores. Returns (out_full, exec_time_ns_or_None)."""
    nc = _get_nc()
    ident, mask, ones = _consts()
    qs = np.ascontiguousarray(np.asarray(q, dtype=np.float32).reshape(B * H, SEQ, D))
    ks = np.ascontiguousarray(np.asarray(k, dtype=np.float32).reshape(B * H, SEQ, D))
    vs = np.ascontiguousarray(np.asarray(v, dtype=np.float32).reshape(B * H, SEQ, D))
    in_maps = []
    for c in range(N_CORES):
        sl = slice(SLICES * c, SLICES * (c + 1))
        in_maps.append(
            {
                "q": qs[sl],
                "k": ks[sl],
                "v": vs[sl],
                "ident": ident,
                "mask": mask,
                "ones": ones,
            }
        )
    res = run_bass_kernel_spmd(nc, in_maps, list(range(N_CORES)), trace=trace)
    outs = [res.results[c]["out"] for c in range(N_CORES)]
    full = np.concatenate(outs, axis=0).reshape(B, H, SEQ, D)
    return full, res.exec_time_ns


def kernel(q, k, v):
    out, _ = run_sharded(q, k, v, trace=False)
    return out


# revision 28
# speedup vs baseline: 1.5807x; 1.0624x over previous
"""Causal multi-head attention on 8 Trainium2 NeuronCores.

Full inputs q/k/v: [2, 16, 2048, 64] f32. The 32 (batch, head) slices are
sharded 4-per-core across 8 cores; each core runs an SPMD Bass/Tile kernel
computing causal softmax(Q K^T / 8) V for its 4 slices.

Self-contained: only needs numpy + the concourse runtime at /opt/trn_rl_repo.
"""

import os
import sys

import numpy as np

for _p in ("/opt/trn_rl_repo",):
    if _p not in sys.path:
        sys.path.insert(0, _p)

from concourse import bass, tile, mybir  # noqa: E402
from concourse.bass_utils import run_bass_kernel_spmd  # noqa: E402

F32 = mybir.dt.float32
F32R = mybir.dt.float32r  # fp32 bits, single-pass PE matmul (4x faster)
BF16 = mybir.dt.bfloat16

B, H, SEQ, D = 2, 16, 2048, 64
N_CORES = 8
SLICES = (B * H) // N_CORES  # 4 per core
NBLK = SEQ // 128  # 16 seq blocks of 128
HALF = 1024  # q processed in halves (PSUM budget)
EXP = mybir.ActivationFunctionType.Exp

NEG_BIG = -1.0e30


def _pieces(start_col):
    """Split [start_col, 1024) at the 512 PSUM-bank boundary."""
    if start_col < 512:
        return [(start_col, 512), (512, 1024)]
    if start_col < 1024:
        return [(start_col, 1024)]
    return []


def _attention_body(ctx, tc, q, k, v, ident, mask, ones, out, slices=SLICES):
    nc = tc.nc

    const_pool = ctx.enter_context(tc.tile_pool(name="const", bufs=1))
    nat_pool = ctx.enter_context(tc.tile_pool(name="nat", bufs=2))
    qkt_pool = ctx.enter_context(tc.tile_pool(name="qkt", bufs=2))
    vt_pool = ctx.enter_context(tc.tile_pool(name="vt", bufs=2))
    pt_pool = ctx.enter_context(tc.tile_pool(name="pt", bufs=3))
    ots_pool = ctx.enter_context(tc.tile_pool(name="ots", bufs=2))
    osb_pool = ctx.enter_context(tc.tile_pool(name="osb", bufs=2))
    rec_pool = ctx.enter_context(tc.tile_pool(name="rec", bufs=2))
    st_psum = ctx.enter_context(tc.tile_pool(name="stp", bufs=2, space="PSUM"))
    ot_psum = ctx.enter_context(tc.tile_pool(name="otp", bufs=1, space="PSUM"))
    tr_psum = ctx.enter_context(tc.tile_pool(name="trp", bufs=2, space="PSUM"))

    ident_sb = const_pool.tile([128, 128], F32)
    nc.sync.dma_start(ident_sb[:, :], ident[:, :])
    mask_sb = const_pool.tile([128, 128], F32)
    nc.sync.dma_start(mask_sb[:, :], mask[:, :])

    for s in range(slices):
        # ---- load this slice ----
        qn = nat_pool.tile([128, NBLK, 64], F32, tag="qn")
        nc.sync.dma_start(qn[:, :, :], q[s].rearrange("(n p) d -> p n d", p=128))
        kn = nat_pool.tile([128, NBLK, 64], F32, tag="kn")
        nc.sync.dma_start(kn[:, :, :], k[s].rearrange("(n p) d -> p n d", p=128))
        vt = vt_pool.tile([128, NBLK, 65], F32R)
        # SWDGE (gpsimd) path: casts f32 -> f32r during the DMA
        nc.gpsimd.dma_start(vt[:, :, 0:64], v[s].rearrange("(n p) d -> p n d", p=128))
        nc.gpsimd.dma_start(vt[:, :, 64:65], ones[:, :, :])  # -> softmax denominator

        # ---- build Q^T, K^T [64, 2048] via PE transposes ----
        # Rows 64:128 hold a replica of rows 0:64 so S^T matmuls can be
        # row-packed two-at-a-time into PE row groups (0,0) and (64,0).
        qt = qkt_pool.tile([128, SEQ], F32R, tag="qt")
        kt = qkt_pool.tile([128, SEQ], F32R, tag="kt")
        for nat, tt in ((qn, qt), (kn, kt)):
            for c in range(4):  # 512-wide chunks
                tp = tr_psum.tile([64, 512], F32, tag="tr")
                for j in range(4):
                    nc.tensor.transpose(
                        tp[:, j * 128 : (j + 1) * 128],
                        nat[:, 4 * c + j, :],
                        ident_sb[:, :],
                    )
                cs = slice(c * 512, (c + 1) * 512)
                nc.vector.tensor_copy(tt[0:64, cs], tp[:, :])
                nc.sync.dma_start(tt[64:128, cs], tt[0:64, cs])

        # ---- main loop: q in two halves ----
        for h in range(2):
            ot = ot_psum.tile([65, HALF], F32, tag="ot")
            kb_hi = 8 if h == 0 else 16
            for kb in range(kb_hi):
                start_col = max(0, 128 * kb - HALF * h)
                pieces = _pieces(start_col)

                # S^T[kb, q] = K[kb] @ Q^T   (contraction over d=64).
                # Two pieces issue into PE row groups 0/64 and run
                # concurrently (each contracts its own 64-partition half).
                st = st_psum.tile([128, HALF], F32, tag="st")
                for i, (a, b) in enumerate(pieces):
                    r = 64 * i
                    nc.tensor.matmul(
                        st[:, a:b],
                        kt[r : r + 64, 128 * kb : 128 * kb + 128],
                        qt[r : r + 64, HALF * h + a : HALF * h + b],
                        start=True,
                        stop=True,
                    )
                # causal mask on the diagonal block
                if 8 * h <= kb < 8 * h + 8:
                    dc = 128 * kb - HALF * h
                    nc.vector.tensor_add(
                        st[:, dc : dc + 128], st[:, dc : dc + 128], mask_sb[:, :]
                    )
                # P^T = exp(S^T / 8)
                pt = pt_pool.tile([128, HALF], F32R, tag="pt")
                nc.scalar.activation(
                    pt[:, start_col:HALF],
                    st[:, start_col:HALF],
                    EXP,
                    bias=0.0,
                    scale=0.125,
                )
                # O^T[d|1, q] += [V[kb] | 1]^T.T @ P^T
                for a, b in pieces:
                    last_kb = 8 * h + 3 if b == 512 else 8 * h + 7
                    nc.tensor.matmul(
                        ot[:, a:b],
                        vt[:, kb, :],
                        pt[:, a:b],
                        start=(kb == 0),
                        stop=(kb == last_kb),
                    )

            # ---- epilogue: transpose back, normalize, store ----
            ots = ots_pool.tile([65, HALF], F32)
            nc.vector.tensor_copy(ots[:, :], ot[:, :])
            outt = st_psum.tile([128, 8, 128], F32, tag="st")
            for j in range(8):
                nc.tensor.transpose(
                    outt[:, j, 0:65],
                    ots[:, j * 128 : (j + 1) * 128],
                    ident_sb[0:65, 0:65],
                )
            rec = rec_pool.tile([128, 8], F32)
            nc.vector.reciprocal(rec[:, :], outt[:, :, 64])
            osb = osb_pool.tile([128, 8, 64], F32)
            nc.vector.tensor_mul(
                osb[:, :, :], outt[:, :, 0:64], rec[:, :].broadcast_to([128, 8, 64])
            )
            nc.sync.dma_start(
                out[s].rearrange("(n p) d -> p n d", p=128)[:, 8 * h : 8 * h + 8, :],
                osb[:, :, :],
            )


def build_nc(slices=SLICES):
    nc = bass.Bass()
    q = nc.declare_dram_parameter("q", [slices, SEQ, D], F32, isOutput=False)
    k = nc.declare_dram_parameter("k", [slices, SEQ, D], F32, isOutput=False)
    v = nc.declare_dram_parameter("v", [slices, SEQ, D], F32, isOutput=False)
    ident = nc.declare_dram_parameter("ident", [128, 128], F32, isOutput=False)
    mask = nc.declare_dram_parameter("mask", [128, 128], F32, isOutput=False)
    ones = nc.declare_dram_parameter("ones", [128, NBLK, 1], F32, isOutput=False)
    out = nc.declare_dram_parameter("out", [slices, SEQ, D], F32, isOutput=True)

    import contextlib

    with tile.TileContext(nc) as tc:
        with contextlib.ExitStack() as ctx:
            _attention_body(
                ctx,
                tc,
                q[:],
                k[:],
                v[:],
                ident[:],
                mask[:],
                ones[:],
                out[:],
                slices=slices,
            )
    # TRN2 allows at most one sync-wait per instruction; Tile can emit more.
    # These bacc legalization passes split the excess onto ldweights /
    # InstEventSemaphore instructions so walrus codegen accepts the BIR.
    import bass_rust

    bass_rust.move_matmul_waits_to_ldweights(nc.m)
    bass_rust.generate_event_semaphores(nc)
    return nc


def _consts():
    ident = np.eye(128, dtype=np.float32)
    i = np.arange(128, dtype=np.int64)
    mask = np.where(i[:, None] <= i[None, :], 0.0, NEG_BIG).astype(np.float32)
    ones = np.ones((128, NBLK, 1), dtype=np.float32)
    return ident, mask, ones


_NC_CACHE = {}


def _get_nc(slices=SLICES):
    if slices not in _NC_CACHE:
        _NC_CACHE[slices] = build_nc(slices)
    return _NC_CACHE[slices]


def run_sharded(q, k, v, trace=False):
    """Run on 8 cores. Returns (out_full, exec_time_ns_or_None)."""
    nc = _get_nc()
    ident, mask, ones = _consts()
    qs = np.ascontiguousarray(np.asarray(q, dtype=np.float32).reshape(B * H, SEQ, D))
    ks = np.ascontiguousarray(np.asarray(k, dtype=np.float32).reshape(B * H, SEQ, D))
    vs = np.ascontiguousarray(np.asarray(v, dtype=np.float32).reshape(B * H, SEQ, D))
    in_maps = []
    for c in range(N_CORES):
        sl = slice(SLICES * c, SLICES * (c + 1))
        in_maps.append(
            {
                "q": qs[sl],
                "k": ks[sl],
                "v": vs[sl],
                "ident": ident,
                "mask": mask,
                "ones": ones,
            }
        )
    res = run_bass_kernel_spmd(nc, in_maps, list(range(N_CORES)), trace=trace)
    outs = [res.results[c]["out"] for c in range(N_CORES)]
    full = np.concatenate(outs, axis=0).reshape(B, H, SEQ, D)
    return full, res.exec_time_ns


def kernel(q, k, v):
    out, _ = run_sharded(q, k, v, trace=False)
    return out


# revision 30
# speedup vs baseline: 1.6683x; 1.0554x over previous
"""Causal multi-head attention on 8 Trainium2 NeuronCores.

Full inputs q/k/v: [2, 16, 2048, 64] f32. The 32 (batch, head) slices are
sharded 4-per-core across 8 cores; each core runs an SPMD Bass/Tile kernel
computing causal softmax(Q K^T / 8) V for its 4 slices.

Self-contained: only needs numpy + the concourse runtime at /opt/trn_rl_repo.
"""

import os
import sys

import numpy as np

for _p in ("/opt/trn_rl_repo",):
    if _p not in sys.path:
        sys.path.insert(0, _p)

from concourse import bass, tile, mybir  # noqa: E402
from concourse.bass_utils import run_bass_kernel_spmd  # noqa: E402

F32 = mybir.dt.float32
F32R = mybir.dt.float32r  # fp32 bits, single-pass PE matmul (4x faster)
BF16 = mybir.dt.bfloat16

B, H, SEQ, D = 2, 16, 2048, 64
N_CORES = 8
SLICES = (B * H) // N_CORES  # 4 per core
NBLK = SEQ // 128  # 16 seq blocks of 128
HALF = 1024  # q processed in halves (PSUM budget)
EXP = mybir.ActivationFunctionType.Exp

NEG_BIG = -1.0e30


def _pieces(start_col):
    """Split [start_col, 1024) at the 512 PSUM-bank boundary."""
    if start_col < 512:
        return [(start_col, 512), (512, 1024)]
    if start_col < 1024:
        return [(start_col, 1024)]
    return []


def _attention_body(ctx, tc, q, k, v, ident, mask, ones, out, slices=SLICES):
    nc = tc.nc

    const_pool = ctx.enter_context(tc.tile_pool(name="const", bufs=1))
    nat_pool = ctx.enter_context(tc.tile_pool(name="nat", bufs=2))
    qkt_pool = ctx.enter_context(tc.tile_pool(name="qkt", bufs=2))
    vt_pool = ctx.enter_context(tc.tile_pool(name="vt", bufs=2))
    pt_pool = ctx.enter_context(tc.tile_pool(name="pt", bufs=3))
    ots_pool = ctx.enter_context(tc.tile_pool(name="ots", bufs=2))
    osb_pool = ctx.enter_context(tc.tile_pool(name="osb", bufs=2))
    rec_pool = ctx.enter_context(tc.tile_pool(name="rec", bufs=2))
    st_psum = ctx.enter_context(tc.tile_pool(name="stp", bufs=2, space="PSUM"))
    ot_psum = ctx.enter_context(tc.tile_pool(name="otp", bufs=1, space="PSUM"))
    tr_psum = ctx.enter_context(tc.tile_pool(name="trp", bufs=2, space="PSUM"))

    ident_sb = const_pool.tile([128, 128], F32)
    nc.sync.dma_start(ident_sb[:, :], ident[:, :])
    mask_sb = const_pool.tile([128, 128], F32)
    nc.sync.dma_start(mask_sb[:, :], mask[:, :])

    for s in range(slices):
        # ---- load this slice ----
        qn = nat_pool.tile([128, NBLK, 64], F32, tag="qn")
        nc.sync.dma_start(qn[:, :, :], q[s].rearrange("(n p) d -> p n d", p=128))
        kn = nat_pool.tile([128, NBLK, 64], F32, tag="kn")
        nc.sync.dma_start(kn[:, :, :], k[s].rearrange("(n p) d -> p n d", p=128))
        vt = vt_pool.tile([128, NBLK, 65], BF16)
        # SWDGE (gpsimd) path: casts f32 -> bf16 during the DMA
        nc.gpsimd.dma_start(vt[:, :, 0:64], v[s].rearrange("(n p) d -> p n d", p=128))
        nc.gpsimd.dma_start(vt[:, :, 64:65], ones[:, :, :])  # -> softmax denominator

        # ---- build Q^T, K^T [64, 2048] via PE transposes ----
        # Rows 64:128 hold a replica of rows 0:64 so S^T matmuls can be
        # row-packed two-at-a-time into PE row groups (0,0) and (64,0).
        qt = qkt_pool.tile([128, SEQ], F32R, tag="qt")
        kt = qkt_pool.tile([128, SEQ], F32R, tag="kt")
        for nat, tt in ((qn, qt), (kn, kt)):
            for c in range(4):  # 512-wide chunks
                tp = tr_psum.tile([64, 512], F32, tag="tr")
                for j in range(4):
                    nc.tensor.transpose(
                        tp[:, j * 128 : (j + 1) * 128],
                        nat[:, 4 * c + j, :],
                        ident_sb[:, :],
                    )
                cs = slice(c * 512, (c + 1) * 512)
                nc.vector.tensor_copy(tt[0:64, cs], tp[:, :])
                nc.sync.dma_start(tt[64:128, cs], tt[0:64, cs])

        # ---- main loop: q in two halves ----
        for h in range(2):
            ot = ot_psum.tile([65, HALF], F32, tag="ot")
            kb_hi = 8 if h == 0 else 16
            for kb in range(kb_hi):
                start_col = max(0, 128 * kb - HALF * h)
                pieces = _pieces(start_col)

                # S^T[kb, q] = K[kb] @ Q^T   (contraction over d=64).
                # Two pieces issue into PE row groups 0/64 and run
                # concurrently (each contracts its own 64-partition half).
                st = st_psum.tile([128, HALF], F32, tag="st")
                for i, (a, b) in enumerate(pieces):
                    r = 64 * i
                    nc.tensor.matmul(
                        st[:, a:b],
                        kt[r : r + 64, 128 * kb : 128 * kb + 128],
                        qt[r : r + 64, HALF * h + a : HALF * h + b],
                        start=True,
                        stop=True,
                    )
                # causal mask on the diagonal block
                if 8 * h <= kb < 8 * h + 8:
                    dc = 128 * kb - HALF * h
                    nc.vector.tensor_add(
                        st[:, dc : dc + 128], st[:, dc : dc + 128], mask_sb[:, :]
                    )
                # P^T = exp(S^T / 8)
                pt = pt_pool.tile([128, HALF], BF16, tag="pt")
                nc.scalar.activation(
                    pt[:, start_col:HALF],
                    st[:, start_col:HALF],
                    EXP,
                    bias=0.0,
                    scale=0.125,
                )
                # O^T[d|1, q] += [V[kb] | 1]^T.T @ P^T
                for a, b in pieces:
                    last_kb = 8 * h + 3 if b == 512 else 8 * h + 7
                    nc.tensor.matmul(
                        ot[:, a:b],
                        vt[:, kb, :],
                        pt[:, a:b],
                        start=(kb == 0),
                        stop=(kb == last_kb),
                    )

            # ---- epilogue: transpose back, normalize, store ----
            ots = ots_pool.tile([65, HALF], F32)
            nc.vector.tensor_copy(ots[:, :], ot[:, :])
            outt = st_psum.tile([128, 8, 128], F32, tag="st")
            for j in range(8):
                nc.tensor.transpose(
                    outt[:, j, 0:65],
                    ots[:, j * 128 : (j + 1) * 128],
                    ident_sb[0:65, 0:65],
                )
            rec = rec_pool.tile([128, 8], F32)
            nc.vector.reciprocal(rec[:, :], outt[:, :, 64])
            osb = osb_pool.tile([128, 8, 64], F32)
            nc.vector.tensor_mul(
                osb[:, :, :], outt[:, :, 0:64], rec[:, :].broadcast_to([128, 8, 64])
            )
            nc.sync.dma_start(
                out[s].rearrange("(n p) d -> p n d", p=128)[:, 8 * h : 8 * h + 8, :],
                osb[:, :, :],
            )


def build_nc(slices=SLICES):
    nc = bass.Bass()
    q = nc.declare_dram_parameter("q", [slices, SEQ, D], F32, isOutput=False)
    k = nc.declare_dram_parameter("k", [slices, SEQ, D], F32, isOutput=False)
    v = nc.declare_dram_parameter("v", [slices, SEQ, D], F32, isOutput=False)
    ident = nc.declare_dram_parameter("ident", [128, 128], F32, isOutput=False)
    mask = nc.declare_dram_parameter("mask", [128, 128], F32, isOutput=False)
    ones = nc.declare_dram_parameter("ones", [128, NBLK, 1], F32, isOutput=False)
    out = nc.declare_dram_parameter("out", [slices, SEQ, D], F32, isOutput=True)

    import contextlib

    with tile.TileContext(nc) as tc:
        with contextlib.ExitStack() as ctx:
            _attention_body(
                ctx,
                tc,
                q[:],
                k[:],
                v[:],
                ident[:],
                mask[:],
                ones[:],
                out[:],
                slices=slices,
            )
    # TRN2 allows at most one sync-wait per instruction; Tile can emit more.
    # These bacc legalization passes split the excess onto ldweights /
    # InstEventSemaphore instructions so walrus codegen accepts the BIR.
    import bass_rust

    bass_rust.move_matmul_waits_to_ldweights(nc.m)
    bass_rust.generate_event_semaphores(nc)
    return nc


def _consts():
    ident = np.eye(128, dtype=np.float32)
    i = np.arange(128, dtype=np.int64)
    mask = np.where(i[:, None] <= i[None, :], 0.0, NEG_BIG).astype(np.float32)
    ones = np.ones((128, NBLK, 1), dtype=np.float32)
    return ident, mask, ones


_NC_CACHE = {}


def _get_nc(slices=SLICES):
    if slices not in _NC_CACHE:
        _NC_CACHE[slices] = build_nc(slices)
    return _NC_CACHE[slices]


def run_sharded(q, k, v, trace=False):
    """Run on 8 cores. Returns (out_full, exec_time_ns_or_None)."""
    nc = _get_nc()
    ident, mask, ones = _consts()
    qs = np.ascontiguousarray(np.asarray(q, dtype=np.float32).reshape(B * H, SEQ, D))
    ks = np.ascontiguousarray(np.asarray(k, dtype=np.float32).reshape(B * H, SEQ, D))
    vs = np.ascontiguousarray(np.asarray(v, dtype=np.float32).reshape(B * H, SEQ, D))
    in_maps = []
    for c in range(N_CORES):
        sl = slice(SLICES * c, SLICES * (c + 1))
        in_maps.append(
            {
                "q": qs[sl],
                "k": ks[sl],
                "v": vs[sl],
                "ident": ident,
                "mask": mask,
                "ones": ones,
            }
        )
    res = run_bass_kernel_spmd(nc, in_maps, list(range(N_CORES)), trace=trace)
    outs = [res.results[c]["out"] for c in range(N_CORES)]
    full = np.concatenate(outs, axis=0).reshape(B, H, SEQ, D)
    return full, res.exec_time_ns


def kernel(q, k, v):
    out, _ = run_sharded(q, k, v, trace=False)
    return out
